# revision 34
# baseline (speedup 1.0000x reference)
"""Causal self-attention (B=2, T=2048, D=1024, H=16, Dh=64) on 8 NeuronCores.

Sharding: tensor-parallel over heads. Core c owns heads {2c, 2c+1}:
  - QKV: computes q/k/v columns c*128:(c+1)*128 of each section.
      q,k are produced transposed (qT/kT: [128 qkv-cols, tokens]) via
      out = w3_slice.T @ x.T matmuls; v is produced in natural layout
      ([tokens, 128 v-cols]) via PE transposes of the vT chunks.
  - Attention: per (batch, q-chunk of 512, k-tile of 128):
      S^T = K_h @ Q_h.T from kT/qT; the two heads' S matmuls sit in
      disjoint 64-row groups of the PE array and run concurrently
      (row-tiled 64x128 mode).  exp on ACT (no max subtraction needed:
      |S*scale| <= ~6), causal mask via in-place affine_select on the
      diagonal tiles (fill=0 after exp), then out^T accumulated as
      V'.T @ P^T where V' = [V | ones]: row 64 of the PSUM accumulator
      is the softmax denominator.
  - Projection: partial out^T = w_proj_slice.T applied per 128-row slice;
      per-core partial [1024, 4096] outputs are summed on the host.

Schedule notes:
  - S psum is double-buffered per single k-tile ([128,1024] x 2) so the
    PE never waits for the exp of the previous tile.  The HAM clock gate
    re-throttles the PE to 1.2 GHz after any >~3.4us idle window, so the
    whole schedule is built to keep PE gaps short.
  - The softmax normalize chain (DVE copy psum->sbuf, custom-DVE
    reciprocal_approx_fast over the full [65,512] tile, DRAM-bounce
    partition broadcast on the gpsimd queue, gpsimd multiply) is fully
    off the PE path; the projection for q-chunk qc is emitted after the
    S/PV matmuls of qc+1 so its aT input is ready by the time the PE
    reaches it.
  - The S matmuls of diagonal k-tiles trim their moving range to the
    non-fully-masked queries (kept >=256 so fp32r stays at 1 cyc/row);
    the in-place affine_select fills the stale region with 0 after exp.
  - x is staged host-side into a chunk-major layout so each x DMA uses
    16KB contiguous descriptors, alternating between the two HWDGE
    queues (sync/scalar); outputs are staged into [128,1024] tiles (4KB
    descriptors) and dispatched on the sync queue (the scalar queue
    would delay exp; DMA dispatch costs ~1us of engine time each).

All matmuls run in float32r (4-byte data, reduced-precision multiply,
1 cycle/row for moving dims >= 256).

TRN2 allows at most one sync-wait per instruction; bacc's
generate_event_semaphores pass splits multi-wait instructions, which is
why the program is built with bacc.Bacc and compiled before dispatch.
"""

import numpy as np

D_MODEL = 1024
B, T = 2, 2048
RC = 128  # per-core qkv columns per q/k/v section == per-core w_proj rows
M = B * T
N_CORES = 8

_prog_cache = {}
_last_results = None  # BassKernelResults of the most recent run (for profiling)


import os

INPLACE_MASK = os.environ.get("K_INPLACE_MASK", "1") == "1"
EXP_TRIM = os.environ.get("K_EXP_TRIM", "0") == "1"
# normalize modes: divide_v (DVE divide), divide_g (gpsimd divide),
# approx_full (custom-DVE reciprocal on full tile + gpsimd mult),
# recip (native reciprocal + gpsimd mult)
NORM_MODE = os.environ.get("K_NORM_MODE", "approx_full")
EXP_PAIR = os.environ.get("K_EXP_PAIR", "0") == "1"


def build_program(Tb=T, use_vbias=False, use_qkbias=False):
    from contextlib import ExitStack

    import concourse.bass as bass
    import concourse.tile as tile
    from concourse import bacc, mybir
    from concourse.tile import add_dep_helper

    f32 = mybir.dt.float32
    f32r = mybir.dt.float32r
    EXP = mybir.ActivationFunctionType.Exp
    MULT = mybir.AluOpType.mult
    DIV = mybir.AluOpType.divide
    IS_GE = mybir.AluOpType.is_ge

    Mb = B * Tb
    mc_per_b = Tb // 512  # x/m chunks of 512 tokens per batch
    n_mc = B * mc_per_b
    n_qc = Tb // 512      # query chunks per batch

    nc = bacc.Bacc("TRN2", target_bir_lowering=False, debug=False)
    # x, chunk-major: [chunk, partition, ktile, token] -> 16KB descriptors
    xc = nc.dram_tensor("xc", [n_mc, 128, 8, 512], f32r, kind="ExternalInput").ap()
    w3 = nc.dram_tensor("w3", [D_MODEL, 3 * RC], f32r, kind="ExternalInput").ap()
    wp = nc.dram_tensor("wp", [RC, D_MODEL], f32r, kind="ExternalInput").ap()
    ident = nc.dram_tensor("ident", [128, 128], f32r, kind="ExternalInput").ap()
    bqk_d = bv_d = None
    if use_qkbias:
        bqk_d = nc.dram_tensor("bqk", [RC, 2], f32, kind="ExternalInput").ap()
    if use_vbias:
        bv_d = nc.dram_tensor("bv", [RC, 1], f32, kind="ExternalInput").ap()
    out_d = nc.dram_tensor("out", [D_MODEL, Mb], f32, kind="ExternalOutput").ap()
    scr_d = nc.dram_tensor("scr", [B * n_qc, 1, 1024], f32).ap()  # recip bounce

    w3_r = w3.rearrange("(kt p) n -> p kt n", p=128)  # [128, 8, 384]

    with tile.TileContext(nc) as tc:
        with ExitStack() as ctx:
            singles = ctx.enter_context(tc.tile_pool(name="singles", bufs=1))
            xpool = ctx.enter_context(tc.tile_pool(name="xpool", bufs=2))
            ptp = ctx.enter_context(
                tc.tile_pool(name="ptp", bufs=2 if EXP_PAIR else 3))
            rcp = ctx.enter_context(tc.tile_pool(name="rcp", bufs=2))
            rbp = ctx.enter_context(tc.tile_pool(name="rbp", bufs=3))
            vtp = ctx.enter_context(tc.tile_pool(name="vtp", bufs=2))
            pvcp = ctx.enter_context(tc.tile_pool(name="pvcp", bufs=3))
            obp = ctx.enter_context(tc.tile_pool(name="obp", bufs=12))
            ps_a = ctx.enter_context(tc.tile_pool(name="ps_a", bufs=2, space="PSUM"))
            ps_s = ctx.enter_context(
                tc.tile_pool(name="ps_s", bufs=1 if EXP_PAIR else 2,
                             space="PSUM"))
            ps_pv = ctx.enter_context(tc.tile_pool(name="ps_pv", bufs=2, space="PSUM"))

            # identity first (tiny), then PE warmup matmuls so the HAM clock
            # gate is released by the time the first x chunk lands
            id_sb = singles.tile([128, 128], f32r, tag="ident")
            nc.sync.dma_start(id_sb, ident)
            wu_ps = ps_a.tile([128, 512], f32, tag="mm")
            for _ in range(48):
                nc.tensor.matmul(wu_ps[:, 0:128], id_sb, id_sb,
                                 start=True, stop=True)

            # weights on the ACT HWDGE ring (before any output stores);
            # x chunks alternate between the SP and ACT rings
            w3_sb = singles.tile([128, 8, 3 * RC], f32r, tag="w3")
            nc.scalar.dma_start(w3_sb, w3_r)
            wp_sb = singles.tile([128, D_MODEL], f32r, tag="wp")
            nc.scalar.dma_start(wp_sb, wp)
            bqk_sb = bv_sb = None
            if use_qkbias:
                bqk_sb = singles.tile([RC, 2], f32, tag="bqk")
                nc.scalar.dma_start(bqk_sb, bqk_d)
            if use_vbias:
                bv_sb = singles.tile([RC, 1], f32, tag="bv")
                nc.scalar.dma_start(bv_sb, bv_d)

            x_tiles = []
            for mc in range(n_mc):
                x_sb = xpool.tile([128, 8, 512], f32r, tag="x")
                eng = nc.sync if mc % 2 == 0 else nc.scalar
                eng.dma_start(x_sb, xc[mc])
                x_tiles.append(x_sb)

            qT, kT, vb, aT = {}, {}, {}, {}
            for b in range(B):
                qT[b] = singles.tile([128, Tb], f32r, tag=f"qT{b}", name=f"qT{b}")
                kT[b] = singles.tile([128, Tb], f32r, tag=f"kT{b}", name=f"kT{b}")
                vb[b] = singles.tile([128, mc_per_b * 4, 130], f32r, tag=f"vb{b}",
                                     name=f"vb{b}")
                aT[b] = singles.tile([128, Tb], f32r, tag=f"aT{b}", name=f"aT{b}")
                # ones columns for the softmax-denominator rows of PV
                nc.vector.memset(vb[b][:, :, 64:65].bitcast(f32), 1.0)
                nc.vector.memset(vb[b][:, :, 129:130].bitcast(f32), 1.0)

            def emit_qkv(b):
                for mci in range(mc_per_b):
                    mc = b * mc_per_b + mci
                    x_sb = x_tiles[mc]
                    vTs = None
                    for nt in range(3):
                        ps = ps_a.tile([128, 512], f32, tag="mm")
                        for kt in range(8):
                            nc.tensor.matmul(
                                ps,
                                w3_sb[:, kt, nt * RC:(nt + 1) * RC],
                                x_sb[:, kt, :],
                                start=(kt == 0), stop=(kt == 7),
                            )
                        if nt < 2:
                            dest = (qT[b] if nt == 0 else kT[b])[
                                :, mci * 512:(mci + 1) * 512]
                            if use_qkbias:
                                nc.vector.tensor_scalar_add(
                                    dest, ps, bqk_sb[:, nt:nt + 1])
                            else:
                                nc.scalar.copy(dest, ps)
                        else:
                            vTs = vtp.tile([128, 512], f32r, tag="vT")
                            nc.scalar.copy(vTs, ps)
                    # transpose vT chunks into natural [tokens, vcol] layout
                    tp = ps_a.tile([128, 512], f32, tag="mm")
                    for ms in range(4):
                        nc.tensor.transpose(
                            tp[:, ms * 128:(ms + 1) * 128].bitcast(f32r),
                            vTs[:, ms * 128:(ms + 1) * 128],
                            id_sb,
                        )
                    for ms in range(4):
                        mt = mci * 4 + ms
                        sl = tp[:, ms * 128:(ms + 1) * 128].bitcast(f32r)
                        nc.vector.tensor_copy(vb[b][:, mt, 0:64], sl[:, 0:64])
                        nc.vector.tensor_copy(vb[b][:, mt, 65:129], sl[:, 64:128])

            def emit_attn_qc(b, qc):
                nkt = (qc + 1) * 4
                pvs = (
                    ps_pv.tile([65, 512], f32, tag="pv", name="pv0"),
                    ps_pv.tile([65, 512], f32, tag="pv", name="pv1"),
                )
                if EXP_PAIR:
                    # two k-tiles share one flat [128,2048] S psum and ONE
                    # exp instruction (ACT per-instruction cost dominates)
                    for g in range(nkt // 2):
                        s = ps_s.tile([128, 2048], f32, tag="s")
                        pt = ptp.tile([128, 2048], f32r, tag="pt")
                        for j in (0, 1):
                            kt = 2 * g + j
                            i = kt - qc * 4
                            trim_s = min(i * 128, 256) if i >= 0 else 0
                            for h in (0, 1):
                                nc.tensor.matmul(
                                    s[:, j * 1024 + h * 512 + trim_s:
                                      j * 1024 + (h + 1) * 512],
                                    kT[b][h * 64:(h + 1) * 64,
                                          kt * 128:(kt + 1) * 128],
                                    qT[b][h * 64:(h + 1) * 64,
                                          qc * 512 + trim_s:(qc + 1) * 512],
                                    start=True, stop=True,
                                )
                        nc.scalar.activation(pt, s, EXP, scale=0.125)
                        for j in (0, 1):
                            kt = 2 * g + j
                            if kt >= qc * 4:
                                for h in (0, 1):
                                    sl = pt[:, j * 1024 + h * 512:
                                            j * 1024 + (h + 1) * 512]
                                    nc.gpsimd.affine_select(
                                        sl, sl,
                                        pattern=[[1, 512]],
                                        compare_op=IS_GE,
                                        fill=0.0,
                                        base=qc * 512 - kt * 128,
                                        channel_multiplier=-1,
                                    )
                        for j in (0, 1):
                            kt = 2 * g + j
                            for h in (0, 1):
                                nc.tensor.matmul(
                                    pvs[h],
                                    vb[b][:, kt, h * 65:(h + 1) * 65],
                                    pt[:, j * 1024 + h * 512:
                                       j * 1024 + (h + 1) * 512],
                                    start=(kt == 0), stop=(kt == nkt - 1),
                                )
                else:
                    for kt in range(nkt):
                        diag = kt >= qc * 4
                        i = kt - qc * 4 if diag else 0
                        trim_s = min(i * 128, 256)  # keep moving dim >= 256
                        s = ps_s.tile([128, 1024], f32, tag="s")
                        for h in (0, 1):
                            nc.tensor.matmul(
                                s[:, h * 512 + trim_s:(h + 1) * 512],
                                kT[b][h * 64:(h + 1) * 64,
                                      kt * 128:(kt + 1) * 128],
                                qT[b][h * 64:(h + 1) * 64,
                                      qc * 512 + trim_s:(qc + 1) * 512],
                                start=True, stop=True,
                            )
                        pt = ptp.tile([128, 1024], f32r, tag="pt")
                        nc.scalar.activation(pt, s, EXP, scale=0.125)
                        if diag:
                            for h in (0, 1):
                                sl = pt[:, h * 512:(h + 1) * 512]
                                nc.gpsimd.affine_select(
                                    sl, sl,
                                    pattern=[[1, 512]],
                                    compare_op=IS_GE,
                                    fill=0.0,
                                    base=qc * 512 - kt * 128,
                                    channel_multiplier=-1,
                                )
                        for h in (0, 1):
                            nc.tensor.matmul(
                                pvs[h],
                                vb[b][:, kt, h * 65:(h + 1) * 65],
                                pt[:, h * 512:(h + 1) * 512],
                                start=(kt == 0), stop=(kt == nkt - 1),
                            )
                # softmax normalize: psum evacuated fast (vector copies) so
                # the PV psum slots recycle; everything after runs off the
                # PE-critical path
                slot = b * n_qc + qc
                pvcs = []
                for h in (0, 1):
                    pvc = pvcp.tile([65, 512], f32, tag="pvc")
                    nc.vector.tensor_copy(pvc, pvs[h])
                    pvcs.append(pvc)
                # stage 1: both heads' reciprocals land in ONE tile so a
                # single scr write + single broadcast serve the whole chunk
                rc_t = rcp.tile([65, 1024], f32, tag="rc", name="rc_t")
                for h in (0, 1):
                    if NORM_MODE == "approx_full":
                        nc.vector.reciprocal_approx_fast(
                            rc_t[:, h * 512:(h + 1) * 512], pvcs[h])
                    else:
                        nc.vector.reciprocal(
                            rc_t[64:65, h * 512:(h + 1) * 512],
                            pvcs[h][64:65, :])
                d1 = nc.sync.dma_start(scr_d[slot], rc_t[64:65, :])
                # stage 2: one partition-broadcast read, then apply per head
                a0 = scr_d[slot]
                rb_t = rbp.tile([64, 1024], f32, tag="rb")
                d2 = nc.gpsimd.dma_start(rb_t, bass.AP(
                    tensor=a0.tensor, offset=a0.offset,
                    ap=[[0, 64], [1, 1024]]))
                add_dep_helper(d2.ins, d1.ins, reason="scr bounce RAW")
                for h in (0, 1):
                    dst = aT[b][h * 64:(h + 1) * 64, qc * 512:(qc + 1) * 512]
                    nc.gpsimd.tensor_tensor(
                        dst, pvcs[h][0:64, :],
                        rb_t[:, h * 512:(h + 1) * 512], op=MULT)
                    if use_vbias:
                        nc.gpsimd.tensor_scalar_add(
                            dst, dst, bv_sb[h * 64:(h + 1) * 64, 0:1])

            ob_tiles = {}

            def emit_proj(b, qc):
                qp, half = qc // 2, qc % 2
                key = (b, qp)
                if key not in ob_tiles:
                    ob_tiles[key] = ([
                        obp.tile([128, 1024], f32, tag="ob", name=f"ob{nt}")
                        for nt in range(8)], set())
                obs, done = ob_tiles[key]
                for nt in range(8):
                    ps = ps_a.tile([128, 512], f32, tag="mm")
                    nc.tensor.matmul(
                        ps,
                        wp_sb[:, nt * 128:(nt + 1) * 128],
                        aT[b][:, qc * 512:(qc + 1) * 512],
                        start=True, stop=True,
                    )
                    dst = obs[nt][:, half * 512:(half + 1) * 512]
                    nc.vector.tensor_copy(dst, ps)
                done.add(half)
                if b == B - 1 and qp == 0:
                    # final pair: store each half immediately so the last
                    # drain is one q-chunk, not two
                    for nt in range(8):
                        nc.sync.dma_start(
                            out_d[nt * 128:(nt + 1) * 128,
                                  b * Tb + qc * 512: b * Tb + (qc + 1) * 512],
                            obs[nt][:, half * 512:(half + 1) * 512],
                        )
                    if len(done) == 2:
                        del ob_tiles[key]
                elif len(done) == 2:
                    # stores dispatch on the SP queue: the ACT queue would
                    # delay the next q-chunk's exp behind the store triggers
                    for nt in range(8):
                        nc.sync.dma_start(
                            out_d[nt * 128:(nt + 1) * 128,
                                  b * Tb + qp * 1024: b * Tb + (qp + 1) * 1024],
                            obs[nt],
                        )
                    del ob_tiles[key]

            emit_qkv(0)
            emit_qkv(1)
            # batch 1 runs its q-chunks in DESCENDING size order: the big
            # qc3 chunk lands right after batch 0's tail (covering its
            # normalize chain + deferred projection), and the kernel ends
            # on the smallest chunk
            def emit_filler(n):
                # dependency-free matmuls that bridge the tail normalize-
                # chain waits so HAM keeps the PE at 2.4 GHz for the final
                # projections
                fp = ps_a.tile([128, 512], f32, tag="mm", name="fill")
                for _ in range(n):
                    nc.tensor.matmul(fp[:, 0:128], id_sb, id_sb,
                                     start=True, stop=True)

            sched = [(0, qc) for qc in range(n_qc)] + \
                    [(1, qc) for qc in reversed(range(n_qc))]
            prev = None
            for i, (b, qc) in enumerate(sched):
                emit_attn_qc(b, qc)
                if i == len(sched) - 1:
                    emit_filler(40)
                if prev is not None:
                    emit_proj(*prev)
                prev = (b, qc)
            emit_filler(56)
            emit_proj(*prev)

    nc.compile()
    return nc


def make_in_maps(x, w_qkv, b_qkv, use_vbias, use_qkbias):
    """Host-side shard prep. Returns per-core input maps (w_proj added later)."""
    Mx = x.shape[0] * x.shape[1]
    xT = np.ascontiguousarray(x.reshape(Mx, D_MODEL).T)  # [1024, 4096]
    # chunk-major: [chunk, partition, ktile, token-in-chunk]
    xcm = np.ascontiguousarray(
        xT.reshape(8, 128, Mx // 512, 512).transpose(2, 1, 0, 3)
    )
    in_maps = []
    for c in range(N_CORES):
        w3c = np.ascontiguousarray(
            np.concatenate(
                [w_qkv[:, s * D_MODEL + c * RC: s * D_MODEL + (c + 1) * RC]
                 for s in range(3)],
                axis=1,
            )
        )
        im = {"xc": xcm, "w3": w3c,
              "ident": np.eye(128, dtype=np.float32)}
        if use_qkbias:
            im["bqk"] = np.ascontiguousarray(
                np.stack(
                    [b_qkv[c * RC:(c + 1) * RC],
                     b_qkv[D_MODEL + c * RC: D_MODEL + (c + 1) * RC]],
                    axis=1,
                )
            )
        if use_vbias:
            im["bv"] = np.ascontiguousarray(
                b_qkv[2 * D_MODEL + c * RC: 2 * D_MODEL + (c + 1) * RC][:, None]
            )
        in_maps.append(im)
    return in_maps


def kernel(x, w_qkv, b_qkv, w_proj, b_proj):
    from concourse.bass_utils import run_bass_kernel_spmd

    x = np.asarray(x, dtype=np.float32)
    w_qkv = np.asarray(w_qkv, dtype=np.float32)
    b_qkv = np.asarray(b_qkv, dtype=np.float32)
    w_proj = np.asarray(w_proj, dtype=np.float32)
    b_proj = np.asarray(b_proj, dtype=np.float32)

    use_vbias = bool(np.any(b_qkv[2 * D_MODEL:]))
    use_qkbias = bool(np.any(b_qkv[:2 * D_MODEL]))
    key = (T, use_vbias, use_qkbias)
    if key not in _prog_cache:
        _prog_cache[key] = build_program(T, use_vbias, use_qkbias)
    nc = _prog_cache[key]

    in_maps = make_in_maps(x, w_qkv, b_qkv, use_vbias, use_qkbias)
    for c in range(N_CORES):
        in_maps[c]["wp"] = np.ascontiguousarray(w_proj[c * RC:(c + 1) * RC, :])

    res = run_bass_kernel_spmd(nc, in_maps, core_ids=list(range(N_CORES)))
    global _last_results
    _last_results = res
    total = res.results[0]["out"].copy()
    for c in range(1, N_CORES):
        total += res.results[c]["out"]
    out = total.T.reshape(B, T, D_MODEL) + b_proj[None, None, :]
    return np.ascontiguousarray(out.astype(np.float32))


# revision 36
# speedup vs baseline: 1.2365x; 1.2365x over previous
"""Causal self-attention (B=2, T=2048, D=1024, H=16, Dh=64) on 8 NeuronCores.

Sharding: tensor-parallel over heads. Core c owns heads {2c, 2c+1}:
  - QKV: computes q/k/v columns c*128:(c+1)*128 of each section.
      q,k are produced transposed (qT/kT: [128 qkv-cols, tokens]) via
      out = w3_slice.T @ x.T matmuls; v is produced in natural layout
      ([tokens, 128 v-cols]) via PE transposes of the vT chunks.
  - Attention: per (batch, q-chunk of 512, k-tile of 128):
      S^T = K_h @ Q_h.T from kT/qT; the two heads' S matmuls sit in
      disjoint 64-row groups of the PE array and run concurrently
      (row-tiled 64x128 mode).  exp on ACT (no max subtraction needed:
      |S*scale| <= ~6), causal mask via in-place affine_select on the
      diagonal tiles (fill=0 after exp), then out^T accumulated as
      V'.T @ P^T where V' = [V | ones]: row 64 of the PSUM accumulator
      is the softmax denominator.
  - Projection: partial out^T = w_proj_slice.T applied per 128-row slice;
      per-core partial [1024, 4096] outputs are summed on the host.

Schedule notes:
  - S psum is double-buffered per single k-tile ([128,1024] x 2) so the
    PE never waits for the exp of the previous tile.  The HAM clock gate
    re-throttles the PE to 1.2 GHz after any >~3.4us idle window, so the
    whole schedule is built to keep PE gaps short.
  - The softmax normalize chain (DVE copy psum->sbuf, custom-DVE
    reciprocal_approx_fast over the full [65,512] tile, one DRAM-bounce
    partition broadcast per chunk on the gpsimd queue, gpsimd multiply)
    is fully off the PE path; the projection for q-chunk qc is emitted
    after the S/PV matmuls of the NEXT chunk so its aT input is ready by
    the time the PE reaches it.  Batch 1 runs its q-chunks largest-first
    so the kernel never idles the PE long enough to re-arm the HAM
    throttle mid-stream, and ends on the smallest chunk.
  - The S matmuls of diagonal k-tiles trim their moving range to the
    non-fully-masked queries (kept >=256 so fp32r stays at 1 cyc/row);
    the in-place affine_select fills the stale region with 0 after exp.
  - x is staged host-side into a chunk-major layout so each x DMA uses
    16KB contiguous descriptors, alternating between the two HWDGE
    queues (sync/scalar); outputs are staged into [128,1024] tiles (4KB
    descriptors) and dispatched on the sync queue (the scalar queue
    would delay exp; DMA dispatch costs ~1us of engine time each).

All matmuls run in float32r (4-byte data, reduced-precision multiply,
1 cycle/row for moving dims >= 256).

TRN2 allows at most one sync-wait per instruction; bacc's
generate_event_semaphores pass splits multi-wait instructions, which is
why the program is built with bacc.Bacc and compiled before dispatch.
"""

import numpy as np

D_MODEL = 1024
B, T = 2, 2048
RC = 128  # per-core qkv columns per q/k/v section == per-core w_proj rows
M = B * T
N_CORES = 8

_prog_cache = {}
_last_results = None  # BassKernelResults of the most recent run (for profiling)


import os

INPLACE_MASK = os.environ.get("K_INPLACE_MASK", "1") == "1"
EXP_TRIM = os.environ.get("K_EXP_TRIM", "0") == "1"
# normalize modes: divide_v (DVE divide), divide_g (gpsimd divide),
# approx_full (custom-DVE reciprocal on full tile + gpsimd mult),
# recip (native reciprocal + gpsimd mult)
NORM_MODE = os.environ.get("K_NORM_MODE", "approx_full")
EXP_PAIR = os.environ.get("K_EXP_PAIR", "0") == "1"


def build_program(Tb=T, use_vbias=False, use_qkbias=False):
    from contextlib import ExitStack

    import concourse.bass as bass
    import concourse.tile as tile
    from concourse import bacc, mybir
    from concourse.tile import add_dep_helper

    f32 = mybir.dt.float32
    f32r = mybir.dt.float32r
    EXP = mybir.ActivationFunctionType.Exp
    MULT = mybir.AluOpType.mult
    DIV = mybir.AluOpType.divide
    IS_GE = mybir.AluOpType.is_ge

    Mb = B * Tb
    mc_per_b = Tb // 512  # x/m chunks of 512 tokens per batch
    n_mc = B * mc_per_b
    n_qc = Tb // 512      # query chunks per batch

    nc = bacc.Bacc("TRN2", target_bir_lowering=False, debug=False)
    # x, chunk-major: [chunk, partition, ktile, token] -> 16KB descriptors
    xc = nc.dram_tensor("xc", [n_mc, 128, 8, 512], f32r, kind="ExternalInput").ap()
    w3 = nc.dram_tensor("w3", [D_MODEL, 3 * RC], f32r, kind="ExternalInput").ap()
    wp = nc.dram_tensor("wp", [RC, D_MODEL], f32r, kind="ExternalInput").ap()
    ident = nc.dram_tensor("ident", [128, 128], f32r, kind="ExternalInput").ap()
    bqk_d = bv_d = None
    if use_qkbias:
        bqk_d = nc.dram_tensor("bqk", [RC, 2], f32, kind="ExternalInput").ap()
    if use_vbias:
        bv_d = nc.dram_tensor("bv", [RC, 1], f32, kind="ExternalInput").ap()
    out_d = nc.dram_tensor("out", [D_MODEL, Mb], f32, kind="ExternalOutput").ap()
    scr_d = nc.dram_tensor("scr", [B * n_qc, 1, 1024], f32).ap()  # recip bounce

    w3_r = w3.rearrange("(kt p) n -> p kt n", p=128)  # [128, 8, 384]

    with tile.TileContext(nc) as tc:
        with ExitStack() as ctx:
            singles = ctx.enter_context(tc.tile_pool(name="singles", bufs=1))
            xpool = ctx.enter_context(tc.tile_pool(name="xpool", bufs=2))
            ptp = ctx.enter_context(
                tc.tile_pool(name="ptp", bufs=2 if EXP_PAIR else 3))
            rcp = ctx.enter_context(tc.tile_pool(name="rcp", bufs=2))
            rbp = ctx.enter_context(tc.tile_pool(name="rbp", bufs=3))
            vtp = ctx.enter_context(tc.tile_pool(name="vtp", bufs=2))
            pvcp = ctx.enter_context(tc.tile_pool(name="pvcp", bufs=3))
            obp = ctx.enter_context(tc.tile_pool(name="obp", bufs=12))
            ps_a = ctx.enter_context(tc.tile_pool(name="ps_a", bufs=2, space="PSUM"))
            ps_s = ctx.enter_context(
                tc.tile_pool(name="ps_s", bufs=1 if EXP_PAIR else 2,
                             space="PSUM"))
            ps_pv = ctx.enter_context(tc.tile_pool(name="ps_pv", bufs=2, space="PSUM"))

            # identity first (tiny), then PE warmup matmuls so the HAM clock
            # gate is released by the time the first x chunk lands
            id_sb = singles.tile([128, 128], f32r, tag="ident")
            nc.sync.dma_start(id_sb, ident)
            wu_ps = ps_a.tile([128, 512], f32, tag="mm")
            for _ in range(48):
                nc.tensor.matmul(wu_ps[:, 0:128], id_sb, id_sb,
                                 start=True, stop=True)

            # weights on the ACT HWDGE ring (before any output stores);
            # x chunks alternate between the SP and ACT rings
            w3_sb = singles.tile([128, 8, 3 * RC], f32r, tag="w3")
            nc.scalar.dma_start(w3_sb, w3_r)
            wp_sb = singles.tile([128, D_MODEL], f32r, tag="wp")
            nc.scalar.dma_start(wp_sb, wp)
            bqk_sb = bv_sb = None
            if use_qkbias:
                bqk_sb = singles.tile([RC, 2], f32, tag="bqk")
                nc.scalar.dma_start(bqk_sb, bqk_d)
            if use_vbias:
                bv_sb = singles.tile([RC, 1], f32, tag="bv")
                nc.scalar.dma_start(bv_sb, bv_d)

            x_tiles = []
            for mc in range(n_mc):
                x_sb = xpool.tile([128, 8, 512], f32r, tag="x")
                eng = nc.sync if mc % 2 == 0 else nc.scalar
                eng.dma_start(x_sb, xc[mc])
                x_tiles.append(x_sb)

            qT, kT, vb, aT = {}, {}, {}, {}
            for b in range(B):
                qT[b] = singles.tile([128, Tb], f32r, tag=f"qT{b}", name=f"qT{b}")
                kT[b] = singles.tile([128, Tb], f32r, tag=f"kT{b}", name=f"kT{b}")
                vb[b] = singles.tile([128, mc_per_b * 4, 130], f32r, tag=f"vb{b}",
                                     name=f"vb{b}")
                aT[b] = singles.tile([128, Tb], f32r, tag=f"aT{b}", name=f"aT{b}")
                # ones columns for the softmax-denominator rows of PV
                nc.vector.memset(vb[b][:, :, 64:65].bitcast(f32), 1.0)
                nc.vector.memset(vb[b][:, :, 129:130].bitcast(f32), 1.0)

            def emit_qkv(b):
                for mci in range(mc_per_b):
                    mc = b * mc_per_b + mci
                    x_sb = x_tiles[mc]
                    vTs = None
                    for nt in range(3):
                        ps = ps_a.tile([128, 512], f32, tag="mm")
                        for kt in range(8):
                            nc.tensor.matmul(
                                ps,
                                w3_sb[:, kt, nt * RC:(nt + 1) * RC],
                                x_sb[:, kt, :],
                                start=(kt == 0), stop=(kt == 7),
                            )
                        if nt < 2:
                            dest = (qT[b] if nt == 0 else kT[b])[
                                :, mci * 512:(mci + 1) * 512]
                            if use_qkbias:
                                nc.vector.tensor_scalar_add(
                                    dest, ps, bqk_sb[:, nt:nt + 1])
                            else:
                                nc.scalar.copy(dest, ps)
                        else:
                            vTs = vtp.tile([128, 512], f32r, tag="vT")
                            nc.scalar.copy(vTs, ps)
                    # transpose vT chunks into natural [tokens, vcol] layout
                    tp = ps_a.tile([128, 512], f32, tag="mm")
                    for ms in range(4):
                        nc.tensor.transpose(
                            tp[:, ms * 128:(ms + 1) * 128].bitcast(f32r),
                            vTs[:, ms * 128:(ms + 1) * 128],
                            id_sb,
                        )
                    for ms in range(4):
                        mt = mci * 4 + ms
                        sl = tp[:, ms * 128:(ms + 1) * 128].bitcast(f32r)
                        nc.vector.tensor_copy(vb[b][:, mt, 0:64], sl[:, 0:64])
                        nc.vector.tensor_copy(vb[b][:, mt, 65:129], sl[:, 64:128])

            def emit_attn_qc(b, qc):
                nkt = (qc + 1) * 4
                pvs = (
                    ps_pv.tile([65, 512], f32, tag="pv", name="pv0"),
                    ps_pv.tile([65, 512], f32, tag="pv", name="pv1"),
                )
                if EXP_PAIR:
                    # two k-tiles share one flat [128,2048] S psum and ONE
                    # exp instruction (ACT per-instruction cost dominates)
                    for g in range(nkt // 2):
                        s = ps_s.tile([128, 2048], f32, tag="s")
                        pt = ptp.tile([128, 2048], f32r, tag="pt")
                        for j in (0, 1):
                            kt = 2 * g + j
                            i = kt - qc * 4
                            trim_s = min(i * 128, 256) if i >= 0 else 0
                            for h in (0, 1):
                                nc.tensor.matmul(
                                    s[:, j * 1024 + h * 512 + trim_s:
                                      j * 1024 + (h + 1) * 512],
                                    kT[b][h * 64:(h + 1) * 64,
                                          kt * 128:(kt + 1) * 128],
                                    qT[b][h * 64:(h + 1) * 64,
                                          qc * 512 + trim_s:(qc + 1) * 512],
                                    start=True, stop=True,
                                )
                        nc.scalar.activation(pt, s, EXP, scale=0.125)
                        for j in (0, 1):
                            kt = 2 * g + j
                            if kt >= qc * 4:
                                for h in (0, 1):
                                    sl = pt[:, j * 1024 + h * 512:
                                            j * 1024 + (h + 1) * 512]
                                    nc.gpsimd.affine_select(
                                        sl, sl,
                                        pattern=[[1, 512]],
                                        compare_op=IS_GE,
                                        fill=0.0,
                                        base=qc * 512 - kt * 128,
                                        channel_multiplier=-1,
                                    )
                        for j in (0, 1):
                            kt = 2 * g + j
                            for h in (0, 1):
                                nc.tensor.matmul(
                                    pvs[h],
                                    vb[b][:, kt, h * 65:(h + 1) * 65],
                                    pt[:, j * 1024 + h * 512:
                                       j * 1024 + (h + 1) * 512],
                                    start=(kt == 0), stop=(kt == nkt - 1),
                                )
                else:
                    for kt in range(nkt):
                        diag = kt >= qc * 4
                        i = kt - qc * 4 if diag else 0
                        trim_s = min(i * 128, 256)  # keep moving dim >= 256
                        s = ps_s.tile([128, 1024], f32, tag="s")
                        for h in (0, 1):
                            nc.tensor.matmul(
                                s[:, h * 512 + trim_s:(h + 1) * 512],
                                kT[b][h * 64:(h + 1) * 64,
                                      kt * 128:(kt + 1) * 128],
                                qT[b][h * 64:(h + 1) * 64,
                                      qc * 512 + trim_s:(qc + 1) * 512],
                                start=True, stop=True,
                            )
                        pt = ptp.tile([128, 1024], f32r, tag="pt")
                        nc.scalar.activation(pt, s, EXP, scale=0.125)
                        if diag:
                            for h in (0, 1):
                                sl = pt[:, h * 512:(h + 1) * 512]
                                nc.gpsimd.affine_select(
                                    sl, sl,
                                    pattern=[[1, 512]],
                                    compare_op=IS_GE,
                                    fill=0.0,
                                    base=qc * 512 - kt * 128,
                                    channel_multiplier=-1,
                                )
                        for h in (0, 1):
                            nc.tensor.matmul(
                                pvs[h],
                                vb[b][:, kt, h * 65:(h + 1) * 65],
                                pt[:, h * 512:(h + 1) * 512],
                                start=(kt == 0), stop=(kt == nkt - 1),
                            )
                # softmax normalize: psum evacuated fast (vector copies) so
                # the PV psum slots recycle; everything after runs off the
                # PE-critical path
                slot = b * n_qc + qc
                pvcs = []
                for h in (0, 1):
                    pvc = pvcp.tile([65, 512], f32, tag="pvc")
                    nc.vector.tensor_copy(pvc, pvs[h])
                    pvcs.append(pvc)
                # stage 1: both heads' reciprocals land in ONE tile so a
                # single scr write + single broadcast serve the whole chunk
                rc_t = rcp.tile([65, 1024], f32, tag="rc", name="rc_t")
                for h in (0, 1):
                    if NORM_MODE == "approx_full":
                        nc.vector.reciprocal_approx_fast(
                            rc_t[:, h * 512:(h + 1) * 512], pvcs[h])
                    else:
                        nc.vector.reciprocal(
                            rc_t[64:65, h * 512:(h + 1) * 512],
                            pvcs[h][64:65, :])
                d1 = nc.sync.dma_start(scr_d[slot], rc_t[64:65, :])
                # stage 2: one partition-broadcast read, then apply per head
                a0 = scr_d[slot]
                rb_t = rbp.tile([64, 1024], f32, tag="rb")
                d2 = nc.gpsimd.dma_start(rb_t, bass.AP(
                    tensor=a0.tensor, offset=a0.offset,
                    ap=[[0, 64], [1, 1024]]))
                add_dep_helper(d2.ins, d1.ins, reason="scr bounce RAW")
                for h in (0, 1):
                    dst = aT[b][h * 64:(h + 1) * 64, qc * 512:(qc + 1) * 512]
                    nc.gpsimd.tensor_tensor(
                        dst, pvcs[h][0:64, :],
                        rb_t[:, h * 512:(h + 1) * 512], op=MULT)
                    if use_vbias:
                        nc.gpsimd.tensor_scalar_add(
                            dst, dst, bv_sb[h * 64:(h + 1) * 64, 0:1])

            ob_tiles = {}

            def emit_proj(b, qc):
                qp, half = qc // 2, qc % 2
                key = (b, qp)
                if key not in ob_tiles:
                    ob_tiles[key] = ([
                        obp.tile([128, 1024], f32, tag="ob", name=f"ob{nt}")
                        for nt in range(8)], set())
                obs, done = ob_tiles[key]
                for nt in range(8):
                    ps = ps_a.tile([128, 512], f32, tag="mm")
                    nc.tensor.matmul(
                        ps,
                        wp_sb[:, nt * 128:(nt + 1) * 128],
                        aT[b][:, qc * 512:(qc + 1) * 512],
                        start=True, stop=True,
                    )
                    dst = obs[nt][:, half * 512:(half + 1) * 512]
                    nc.vector.tensor_copy(dst, ps)
                done.add(half)
                if b == B - 1 and qp == 0:
                    # final pair: store each half immediately so the last
                    # drain is one q-chunk, not two
                    for nt in range(8):
                        nc.sync.dma_start(
                            out_d[nt * 128:(nt + 1) * 128,
                                  b * Tb + qc * 512: b * Tb + (qc + 1) * 512],
                            obs[nt][:, half * 512:(half + 1) * 512],
                        )
                    if len(done) == 2:
                        del ob_tiles[key]
                elif len(done) == 2:
                    # stores dispatch on the SP queue: the ACT queue would
                    # delay the next q-chunk's exp behind the store triggers
                    for nt in range(8):
                        nc.sync.dma_start(
                            out_d[nt * 128:(nt + 1) * 128,
                                  b * Tb + qp * 1024: b * Tb + (qp + 1) * 1024],
                            obs[nt],
                        )
                    del ob_tiles[key]

            emit_qkv(0)
            emit_qkv(1)
            # batch 1 runs its q-chunks in DESCENDING size order: the big
            # qc3 chunk lands right after batch 0's tail (covering its
            # normalize chain + deferred projection), and the kernel ends
            # on the smallest chunk
            sched = [(0, qc) for qc in range(n_qc)] + \
                    [(1, qc) for qc in reversed(range(n_qc))]
            prev = None
            for b, qc in sched:
                emit_attn_qc(b, qc)
                if prev is not None:
                    emit_proj(*prev)
                prev = (b, qc)
            emit_proj(*prev)

    nc.compile()
    return nc


def make_in_maps(x, w_qkv, b_qkv, use_vbias, use_qkbias):
    """Host-side shard prep. Returns per-core input maps (w_proj added later)."""
    Mx = x.shape[0] * x.shape[1]
    xT = np.ascontiguousarray(x.reshape(Mx, D_MODEL).T)  # [1024, 4096]
    # chunk-major: [chunk, partition, ktile, token-in-chunk]
    xcm = np.ascontiguousarray(
        xT.reshape(8, 128, Mx // 512, 512).transpose(2, 1, 0, 3)
    )
    in_maps = []
    for c in range(N_CORES):
        w3c = np.ascontiguousarray(
            np.concatenate(
                [w_qkv[:, s * D_MODEL + c * RC: s * D_MODEL + (c + 1) * RC]
                 for s in range(3)],
                axis=1,
            )
        )
        im = {"xc": xcm, "w3": w3c,
              "ident": np.eye(128, dtype=np.float32)}
        if use_qkbias:
            im["bqk"] = np.ascontiguousarray(
                np.stack(
                    [b_qkv[c * RC:(c + 1) * RC],
                     b_qkv[D_MODEL + c * RC: D_MODEL + (c + 1) * RC]],
                    axis=1,
                )
            )
        if use_vbias:
            im["bv"] = np.ascontiguousarray(
                b_qkv[2 * D_MODEL + c * RC: 2 * D_MODEL + (c + 1) * RC][:, None]
            )
        in_maps.append(im)
    return in_maps


def kernel(x, w_qkv, b_qkv, w_proj, b_proj):
    from concourse.bass_utils import run_bass_kernel_spmd

    x = np.asarray(x, dtype=np.float32)
    w_qkv = np.asarray(w_qkv, dtype=np.float32)
    b_qkv = np.asarray(b_qkv, dtype=np.float32)
    w_proj = np.asarray(w_proj, dtype=np.float32)
    b_proj = np.asarray(b_proj, dtype=np.float32)

    use_vbias = bool(np.any(b_qkv[2 * D_MODEL:]))
    use_qkbias = bool(np.any(b_qkv[:2 * D_MODEL]))
    key = (T, use_vbias, use_qkbias)
    if key not in _prog_cache:
        _prog_cache[key] = build_program(T, use_vbias, use_qkbias)
    nc = _prog_cache[key]

    in_maps = make_in_maps(x, w_qkv, b_qkv, use_vbias, use_qkbias)
    for c in range(N_CORES):
        in_maps[c]["wp"] = np.ascontiguousarray(w_proj[c * RC:(c + 1) * RC, :])

    res = run_bass_kernel_spmd(nc, in_maps, core_ids=list(range(N_CORES)))
    global _last_results
    _last_results = res
    total = res.results[0]["out"].copy()
    for c in range(1, N_CORES):
        total += res.results[c]["out"]
    out = total.T.reshape(B, T, D_MODEL) + b_proj[None, None, :]
    return np.ascontiguousarray(out.astype(np.float32))
